# revision 1
# baseline (speedup 1.0000x reference)
"""Trainium2 Bass kernel for nn_AttrsEncoderLayers (gnn_message_passing).

Math (from the reference):
  h0 = concat(node_attr[src], edge_attr)        [E, 80]
  h1 = relu(BN1(BN0(h0) @ W1))                  [E, 128]
  x  = h1 @ Wg ; a_src = x@att_src ; a_dst = x@att_dst
  dense 6x6 softmax attention within each node's 6-edge group (incl. self-loop)
  h3[n] = sum_{d in g(n)} sum_s alpha[d,s] x[s]   -> BNf(h3)

Structure facts (deterministic in setup_inputs): src = repeat(arange(N), 6);
index_2step = all ordered pairs of distinct edges sharing a source node plus
self loops => attention neighborhood of edge d is exactly its 6-edge group.

Device layout: feature-major (features on SBUF partitions, edges on the free
dim). Host does layout marshaling only (transposes/shard/pack); all arithmetic
that depends on tensor *values* happens on the 8 NeuronCores. BatchNorm shift
terms cancel algebraically (BN is shift invariant), so only the scale of BN0
is needed before W1, and gat_bias cancels in BNf's mean subtraction.

Per core: 2500 nodes, 15000 edges. Cross-core: 3 tiny AllGathers for the
global BN statistics (BN0, BN1, BNf).
"""
import sys
import types

for _p in ("/opt/trn_rl_repo", "/root/.axon_site/_ro/trn_rl_repo"):
    if _p not in sys.path:
        sys.path.insert(0, _p)

import numpy as np
import concourse.bass as bass
import concourse.tile as tile
from concourse import bacc, mybir
from concourse import bass_utils

# ---------------------------------------------------------------- constants
NCORES = 8
NN_G, DEG = 20000, 6
EE_G = NN_G * DEG              # 120000
NN = NN_G // NCORES            # 2500 nodes per core
EE = NN * DEG                  # 15000 edges per core
DN, DE, DIN = 64, 16, 80
HID = 128
OUT = 128
EPS = 1e-5
F32 = mybir.dt.float32
F32R = mybir.dt.float32r
ALU = mybir.AluOpType
ACTF = mybir.ActivationFunctionType

MM1_CH = 480                   # mm1 chunk (80 groups, <=512, x6)
AMM_CH = 1024                  # a-matmul psum chunk
RELU_CH = 3000                 # BN1-apply chunk
CMB_CH = 3000                  # combine chunk: 500 groups, 25 wp-partitions
NPW = 125                      # group-major partitions (20 groups each)
GPP = 20                       # groups per partition in w' layout
RG = [list(range(NCORES))]

_CACHE = {}
LAST_RESULTS = None
import os as _os
KSTAGE = int(_os.environ.get("KSTAGE", "7"))

if not getattr(bass_utils, "_ldwopt_patched", False):
    bass_utils._ldwopt_patched = True
    _orig_walrus_args = bass_utils.get_walrus_args

    def _walrus_args_ldwopt(*a, **k):
        return [x.replace("--enable-ldw-opt=false", "--enable-ldw-opt=true")
                for x in _orig_walrus_args(*a, **k)]

    bass_utils.get_walrus_args = _walrus_args_ldwopt


def _install_ntff_hook():
    """Register the axon NTFF profiling hook under the name bass_utils expects.

    Harmless if profiling is never requested; lets BASS_TRACE=1 produce
    exec_time_ns under axon."""
    try:
        import antenv.axon_hooks  # noqa: F401
        return
    except ImportError:
        pass
    try:
        import trn_agent_boot.trn_boot as tb
        hook = tb._ntff_profile_via_ctypes("/opt/axon/libaxon_pjrt.so")
    except Exception:
        hook = None
    mod_antenv = sys.modules.get("antenv") or types.ModuleType("antenv")
    mod_hooks = types.ModuleType("antenv.axon_hooks")
    _reg = {"hook": hook}
    mod_hooks.set_axon_ntff_profile_hook = lambda h: _reg.__setitem__("hook", h)
    mod_hooks.get_axon_ntff_profile_hook = lambda: _reg["hook"]
    mod_antenv.axon_hooks = mod_hooks
    sys.modules.setdefault("antenv", mod_antenv)
    sys.modules["antenv.axon_hooks"] = mod_hooks


def _r(ap):
    return ap


def _bn_scale_mu(nc, sb, S, gDvec, divisor, tag):
    """From S=[P,2] (sum, sumsq) over `divisor` rows: return (scale, S[:,0:1]).

    gDvec must be g*divisor (host pre-scaled). scale = g/sqrt(var+eps).
    Identity: divisor^2*var = divisor*sumsq - sum^2, so
    scale = g*divisor / sqrt(divisor*sumsq - sum^2 + eps*divisor^2)."""
    P = S.shape[0]
    q = sb.tile([P, 1], F32, tag=f"{tag}_q")
    nc.vector.tensor_tensor(q[:], S[:, 0:1], S[:, 0:1], ALU.mult)
    vD2 = sb.tile([P, 1], F32, tag=f"{tag}_vD2")
    nc.vector.scalar_tensor_tensor(vD2[:], S[:, 1:2], float(divisor), q[:],
                                   ALU.mult, ALU.subtract)
    eps = sb.tile([P, 1], F32, tag=f"{tag}_eps")
    nc.vector.memset(eps[:], EPS * divisor * divisor)
    sdD = sb.tile([P, 1], F32, tag=f"{tag}_sd")
    nc.scalar.activation(sdD[:], vD2[:], ACTF.Sqrt, bias=eps[:])
    rsd = sb.tile([P, 1], F32, tag=f"{tag}_rsd")
    nc.vector.reciprocal(rsd[:], sdD[:])
    sc = sb.tile([P, 1], F32, tag=f"{tag}_sc")
    nc.vector.tensor_tensor(sc[:], gDvec, rsd[:], ALU.mult)
    return sc, S[:, 0:1]


def _allgather_stats(nc, sb, dram, P, tag):
    """Allocate AG bounce buffers for [P,2] stats; returns (ag_in, finish).

    Caller DMAs partial sums into ag_in ([P,2] DRAM), then calls finish()
    which runs the AllGather and returns S=[P,2] (summed over all cores)."""
    ag_in = dram.tile([P, 2], F32, tag=f"{tag}_in")
    ag_out = dram.tile([NCORES * P, 2], F32, tag=f"{tag}_out")

    def finish():
        nc.gpsimd.collective_compute(
            "AllGather", ALU.bypass, replica_groups=RG,
            ins=[ag_in[:].opt()], outs=[ag_out[:].opt()],
        )
        agv = sb.tile([P, 16], F32, tag=f"{tag}_agv")
        nc.sync.dma_start(
            agv[:].rearrange("p (r c) -> p r c", r=NCORES),
            ag_out[:].rearrange("(r p) c -> p r c", r=NCORES),
        )
        S = sb.tile([P, 2], F32, tag=f"{tag}_S")
        nc.vector.tensor_reduce(
            S[:], agv[:].rearrange("p (r c) -> p c r", r=NCORES),
            axis=mybir.AxisListType.X, op=ALU.add,
        )
        return S

    return ag_in, finish


def build(stage=None):
    if stage is None:
        stage = KSTAGE
    nc = bacc.Bacc("TRN2", target_bir_lowering=False, debug=False,
                   num_devices=NCORES)

    nT_d = nc.dram_tensor("nT", [DN, NN], F32R, kind="ExternalInput").ap()
    eT_d = nc.dram_tensor("eT", [DE, EE], F32R, kind="ExternalInput").ap()
    esv_d = nc.dram_tensor("esv", [128, EE * DE // 128], F32, kind="ExternalInput").ap()
    W1_d = nc.dram_tensor("W1", [DIN, HID], F32, kind="ExternalInput").ap()
    vavd_d = nc.dram_tensor("vavd", [HID, 2], F32R, kind="ExternalInput").ap()
    Wg_d = nc.dram_tensor("Wg", [HID, OUT], F32R, kind="ExternalInput").ap()
    bn0_d = nc.dram_tensor("bn0", [DIN, 2], F32, kind="ExternalInput").ap()
    bn1_d = nc.dram_tensor("bn1", [HID, 2], F32, kind="ExternalInput").ap()
    bnf_d = nc.dram_tensor("bnf", [OUT, 2], F32, kind="ExternalInput").ap()
    y_d = nc.dram_tensor("y", [OUT, NN], F32, kind="ExternalOutput").ap()

    ESV_W = EE * DE // 128     # 1875

    def body(tc, sb, sb2, dram, ps, psone):
        # ---------------- loads
        nT = sb.tile([DN, NN], F32R, tag="t_nT")
        nc.sync.dma_start(nT[:], nT_d)
        esv = sb.tile([128, ESV_W], F32, tag="t_esv")
        nc.sync.dma_start(esv[:], esv_d)
        h0T = sb.tile([DIN, EE], F32R, tag="t_band")
        nc.sync.dma_start(h0T[DN:DIN, :], eT_d)
        W1 = sb.tile([DIN, HID], F32, tag="t_W1")
        nc.sync.dma_start(W1[:], W1_d)
        vavd = sb.tile([HID, 2], F32R, tag="t_vavd")
        nc.sync.dma_start(vavd[:], vavd_d)
        Wg = sb.tile([HID, OUT], F32R, tag="t_Wg")
        nc.sync.dma_start(Wg[:], Wg_d)
        bn0 = sb.tile([DIN, 2], F32, tag="t_bn0")
        nc.sync.dma_start(bn0[:], bn0_d)
        bn1 = sb.tile([HID, 2], F32, tag="t_bn1")
        nc.sync.dma_start(bn1[:], bn1_d)
        bnf = sb.tile([OUT, 2], F32, tag="t_bnf")
        nc.sync.dma_start(bnf[:], bnf_d)

        # ---------------- BN0 local stats (sum, sumsq as 2 columns)
        scrap0 = sb.tile([128, NN], F32, tag="t_s0h3")
        pn = sb.tile([DN, 2], F32, tag="t_pn")
        nc.vector.tensor_reduce(pn[:, 0:1], nT[:], axis=mybir.AxisListType.X, op=ALU.add)
        nc.scalar.activation(scrap0[0:DN, :], nT[:], ACTF.Square, accum_out=pn[:, 1:2])
        pn6 = sb.tile([DN, 2], F32, tag="t_pn6")
        nc.scalar.mul(pn6[:], pn[:], float(DEG))

        pe = sb.tile([128, 2], F32, tag="t_pe")
        nc.vector.tensor_reduce(pe[:, 0:1], esv[:], axis=mybir.AxisListType.X, op=ALU.add)
        nc.scalar.activation(scrap0[:, 0:ESV_W], esv[:], ACTF.Square, accum_out=pe[:, 1:2])
        # fold 8 blocks of 16 (esv partition p = j*16+f); engines need
        # equal input base partitions, so stage the high half via DMA
        ha = sb.tile([64, 2], F32, tag="t_ha")
        nc.sync.dma_start(ha[:], pe[64:128, :])
        ea = sb.tile([64, 2], F32, tag="t_ea")
        nc.vector.tensor_tensor(ea[:], pe[0:64, :], ha[:], ALU.add)
        hb = sb.tile([32, 2], F32, tag="t_hb")
        nc.sync.dma_start(hb[:], ea[32:64, :])
        eb = sb.tile([32, 2], F32, tag="t_eb")
        nc.vector.tensor_tensor(eb[:], ea[0:32, :], hb[:], ALU.add)
        ec = sb.tile([16, 2], F32, tag="t_ec")
        nc.sync.dma_start(ec[:], eb[16:32, :])
        sE = sb.tile([16, 2], F32, tag="t_sE")
        nc.vector.tensor_tensor(sE[:], eb[0:16, :], ec[:], ALU.add)

        ag1_in, ag1_fin = _allgather_stats(nc, sb, dram, DIN, "ag1")
        nc.sync.dma_start(ag1_in[0:DN, :], pn6[:])
        nc.sync.dma_start(ag1_in[DN:DIN, :], sE[:])
        S0 = ag1_fin()
        s0v, _mu0 = _bn_scale_mu(nc, sb, S0, bn0[:, 0:1], EE_G, "b0")

        W1p = sb.tile([DIN, HID], F32R, tag="t_W1p")
        nc.vector.tensor_scalar(W1p[:], W1[:], s0v[:], None, ALU.mult)

        if stage < 2:
            outsb = sb.tile([128, NN], F32, tag="t_nT")
            nc.vector.memset(outsb[:], 0.0)
            nc.vector.tensor_copy(outsb[0:80, 0:1], s0v[:])
            nc.sync.dma_start(y_d, outsb[:])
            return
        # node part of h0T: each node column repeated 6x (runs during AG#1 idle)
        nvrep = nT[:].unsqueeze(2).broadcast_to([DN, NN, DEG])
        for e0 in range(0, EE, RELU_CH):
            ch = min(RELU_CH, EE - e0)
            g0, ng = e0 // DEG, ch // DEG
            nc.vector.tensor_scalar(
                h0T[0:DN, e0:e0 + ch].rearrange("p (g s) -> p g s", s=DEG),
                nvrep[:, g0:g0 + ng, :], 1.0, None, ALU.mult)

        ones1 = sb.tile([1, 128], F32, tag="t_ones1")
        nc.vector.memset(ones1[:], 1.0)
        ones_r = sb.tile([1, 128], F32R, tag="t_onesr")
        nc.vector.tensor_scalar(ones_r[:], ones1[:], 1.0, None, ALU.mult)


        # ---------------- mm1 + BN1 stats
        h1pre = sb.tile([128, EE], F32, tag="t_h1")
        n_ch1 = (EE + 1023) // 1024
        accS = sb.tile([128, n_ch1], F32, tag="t_accS")
        accQ = sb.tile([128, n_ch1], F32, tag="t_accQ")
        for k in range(n_ch1):
            e0 = k * 1024
            ch = min(1024, EE - e0)
            psB = ps.tile([128, 1024], F32, tag="psmain")
            for i0 in range(0, ch, 512):
                w_ = min(512, ch - i0)
                nc.tensor.matmul(psB[:, i0:i0 + w_], _r(W1p[:]),
                                 _r(h0T[:, e0 + i0:e0 + i0 + w_]),
                                 start=True, stop=True)
            nc.scalar.activation(h1pre[:, e0:e0 + ch], psB[:, 0:ch],
                                 ACTF.Copy, accum_out=accS[:, k:k + 1])
            pscr = sb2.tile([128, 1024], F32, tag="t_wh1")
            nc.vector.scalar_tensor_tensor(
                pscr[:, 0:ch], h1pre[:, e0:e0 + ch], 1.0, h1pre[:, e0:e0 + ch],
                ALU.mult, ALU.mult, accum_out=accQ[:, k:k + 1])

        sum1 = sb.tile([128, 1], F32, tag="t_sum1")
        nc.vector.tensor_reduce(sum1[:], accS[:], axis=mybir.AxisListType.X, op=ALU.add)
        ssq1 = sb.tile([128, 1], F32, tag="t_ssq1")
        nc.vector.tensor_reduce(ssq1[:], accQ[:], axis=mybir.AxisListType.X, op=ALU.add)
        if stage < 3:
            outsb = sb.tile([128, NN], F32, tag="t_nT")
            nc.vector.memset(outsb[:], 0.0)
            nc.vector.tensor_copy(outsb[:, 0:1], sum1[:])
            nc.vector.tensor_copy(outsb[:, 1:2], ssq1[:])
            nc.sync.dma_start(y_d, outsb[:])
            return
        ag2_in, ag2_fin = _allgather_stats(nc, sb, dram, HID, "ag2")
        nc.sync.dma_start(ag2_in[:, 0:1], sum1[:])
        nc.sync.dma_start(ag2_in[:, 1:2], ssq1[:])
        S1 = ag2_fin()
        s1v, sum1g = _bn_scale_mu(nc, sb, S1, bn1[:, 0:1], EE_G, "b1")
        t1 = sb.tile([128, 1], F32, tag="t_t1")
        nc.vector.tensor_tensor(t1[:], sum1g, s1v[:], ALU.mult)
        b1e = sb.tile([128, 1], F32, tag="t_b1e")
        nc.vector.scalar_tensor_tensor(b1e[:], t1[:], -1.0 / EE_G, bn1[:, 1:2],
                                       ALU.mult, ALU.add)

        # -------- BN1 apply + ReLU interleaved with a-matmul + per-block folds
        h1 = sb.tile([128, EE], F32R, tag="t_band")
        a_sb = sb.tile([2, EE], F32, tag="t_h1")
        asrc = sb.tile([NPW, GPP * DEG], F32, tag="t_asrc")
        adst = sb.tile([NPW, GPP * DEG], F32, tag="t_adst")
        NPB = RELU_CH // (GPP * DEG)
        for bi, e0 in enumerate(range(0, EE, RELU_CH)):
            ch = min(RELU_CH, EE - e0)
            nc.scalar.activation(h1[:, e0:e0 + ch], h1pre[:, e0:e0 + ch],
                                 ACTF.Relu, bias=b1e[:], scale=s1v[:])
            for a0 in range(e0, e0 + ch, 1500):
                psA = psone.tile([2, 1536], F32, tag="psA")
                for i0, w in ((0, 512), (512, 512), (1024, 476)):
                    nc.tensor.matmul(psA[:, i0:i0 + w], _r(vavd[:]),
                                     _r(h1[:, a0 + i0:a0 + i0 + w]),
                                     start=True, stop=True)
                if (a0 // 1500) % 2 == 0:
                    nc.vector.tensor_scalar(a_sb[:, a0:a0 + 1500], psA[:, 0:1500],
                                            1.0, None, ALU.mult)
                else:
                    nc.scalar.copy(a_sb[:, a0:a0 + 1500], psA[:, 0:1500])
            p0 = bi * NPB
            nc.sync.dma_start(asrc[p0:p0 + NPB, :], a_sb[0:1, e0:e0 + ch])
            nc.sync.dma_start(adst[p0:p0 + NPB, :], a_sb[1:2, e0:e0 + ch])

        if stage < 4:
            outsb = sb.tile([128, NN], F32, tag="t_nT")
            nc.vector.memset(outsb[:], 0.0)
            nc.vector.tensor_reduce(outsb[:, 0:1], h1[:, 0:3000].bitcast(F32),
                                    axis=mybir.AxisListType.X, op=ALU.add)
            nc.sync.dma_start(y_d, outsb[:])
            return

        if stage < 5:
            outsb = sb.tile([128, NN], F32, tag="t_nT")
            nc.vector.memset(outsb[:], 0.0)
            nc.vector.tensor_reduce(outsb[0:125, 0:1], asrc[:],
                                    axis=mybir.AxisListType.X, op=ALU.add)
            nc.vector.tensor_reduce(outsb[0:125, 1:2], adst[:],
                                    axis=mybir.AxisListType.X, op=ALU.add)
            nc.sync.dma_start(y_d, outsb[:])
            return
        # ---------------- dense 6x6 group attention -> per-edge weight w
        L = sb.tile([NPW, GPP * 36], F32, tag="t_L")
        asrc_v = asrc[:].rearrange("p (t s) -> p t s", s=DEG).unsqueeze(2) \
            .broadcast_to([NPW, GPP, DEG, DEG])
        adst_v = adst[:].rearrange("p (t d) -> p t d", d=DEG).unsqueeze(3) \
            .broadcast_to([NPW, GPP, DEG, DEG])
        nc.vector.tensor_tensor(
            L[:].rearrange("p (t d s) -> p t d s", d=DEG, s=DEG),
            asrc_v, adst_v, ALU.add)
        nc.vector.scalar_tensor_tensor(L[:], L[:], 0.2, L[:], ALU.mult, ALU.max)
        nc.scalar.activation(L[:], L[:], ACTF.Exp)
        R = sb.tile([NPW, GPP * DEG], F32, tag="t_R")
        nc.vector.tensor_reduce(
            R[:], L[:].rearrange("p (t d s) -> p t d s", d=DEG, s=DEG),
            axis=mybir.AxisListType.X, op=ALU.add)
        Rinv = sb.tile([NPW, GPP * DEG], F32, tag="t_Rinv")
        nc.vector.reciprocal(Rinv[:], R[:])
        Q = sb.tile([NPW, GPP * 36], F32, tag="t_Q")
        rinv_v = Rinv[:].rearrange("p (t d) -> p t d", d=DEG).unsqueeze(2) \
            .broadcast_to([NPW, GPP, DEG, DEG])
        nc.vector.tensor_tensor(
            Q[:].rearrange("p (t s d) -> p t s d", s=DEG, d=DEG),
            L[:].rearrange("p (t d s) -> p t s d", d=DEG, s=DEG),
            rinv_v, ALU.mult)
        wp = sb.tile([NPW, GPP * DEG], F32, tag="t_wp")
        nc.vector.tensor_reduce(
            wp[:], Q[:].rearrange("p (t s d) -> p t s d", s=DEG, d=DEG),
            axis=mybir.AxisListType.X, op=ALU.add)

        if stage < 6:
            outsb = sb.tile([128, NN], F32, tag="t_nT")
            nc.vector.memset(outsb[:], 0.0)
            nc.vector.tensor_reduce(outsb[0:125, 0:1], wp[:],
                                    axis=mybir.AxisListType.X, op=ALU.add)
            nc.sync.dma_start(y_d, outsb[:])
            return
        # ---------------- combine: y_feat = Wg^T @ sum_s (w * h1) per group
        n_cmb = (EE + CMB_CH - 1) // CMB_CH
        accF = sb.tile([128, n_cmb], F32, tag="t_accF")
        accFq = sb.tile([128, n_cmb], F32, tag="t_accFq")
        h3sb = sb.tile([128, NN], F32, tag="t_s0h3")
        for c in range(n_cmb):
            e0 = c * CMB_CH
            ch = min(CMB_CH, EE - e0)
            ngr = ch // DEG
            p0 = e0 // (GPP * DEG)
            npp = ch // (GPP * DEG)
            wline = sb.tile([1, CMB_CH], F32, tag="t_wline")
            nc.sync.dma_start(wline[:, 0:ch], wp[p0:p0 + npp, :])
            wh1 = sb2.tile([128, CMB_CH], F32R, tag="t_wh1")
            if c % 2 == 0:
                wrep = sb.tile([128, CMB_CH], F32, tag="t_wrep")
                nc.gpsimd.partition_broadcast(wrep[:, 0:ch], wline[:, 0:ch])
                nc.vector.tensor_tensor(wh1[:, 0:ch], h1[:, e0:e0 + ch],
                                        wrep[:, 0:ch].bitcast(F32R), ALU.mult)
            else:
                for i0 in range(0, ch, 500):
                    w_ = min(500, ch - i0)
                    pw = psone.tile([128, 500], F32, tag="pswrep")
                    nc.tensor.matmul(pw[:, 0:w_], ones_r[:],
                                     wline[:, i0:i0 + w_].bitcast(F32R),
                                     start=True, stop=True)
                    nc.vector.tensor_tensor(wh1[:, i0:i0 + w_],
                                            h1[:, e0 + i0:e0 + i0 + w_],
                                            pw[:, 0:w_].bitcast(F32R), ALU.mult)
            h3ps = ps.tile([128, 512], F32, tag="psmain")
            wv = wh1[:].rearrange("p (g s) -> p s g", s=DEG)
            for s in range(DEG):
                nc.tensor.matmul(h3ps[:, 0:ngr], _r(Wg[:]), _r(wv[:, s, 0:ngr]),
                                 start=(s == 0), stop=(s == DEG - 1))
            g0 = e0 // DEG
            nc.scalar.activation(h3sb[:, g0:g0 + ngr], h3ps[:, 0:ngr],
                                 ACTF.Copy, accum_out=accF[:, c:c + 1])
            pscr2 = sb2.tile([128, CMB_CH // DEG], F32, tag="t_scr2")
            nc.vector.scalar_tensor_tensor(
                pscr2[:, 0:ngr], h3sb[:, g0:g0 + ngr], 1.0, h3sb[:, g0:g0 + ngr],
                ALU.mult, ALU.mult, accum_out=accFq[:, c:c + 1])

        if stage < 7:
            outsb = sb.tile([128, NN], F32, tag="t_nT")
            nc.vector.memset(outsb[:], 0.0)
            nc.vector.tensor_copy(outsb[:, 0:n_cmb], accF[:])
            nc.sync.dma_start(y_d, outsb[:])
            return
        # ---------------- BNf
        sumf = sb.tile([128, 1], F32, tag="t_sumf")
        nc.vector.tensor_reduce(sumf[:], accF[:], axis=mybir.AxisListType.X, op=ALU.add)
        ssqf = sb.tile([128, 1], F32, tag="t_ssqf")
        nc.vector.tensor_reduce(ssqf[:], accFq[:], axis=mybir.AxisListType.X, op=ALU.add)
        agf_in, agf_fin = _allgather_stats(nc, sb, dram, OUT, "agf")
        nc.sync.dma_start(agf_in[:, 0:1], sumf[:])
        nc.sync.dma_start(agf_in[:, 1:2], ssqf[:])
        Sf = agf_fin()
        sfv, sumfg = _bn_scale_mu(nc, sb, Sf, bnf[:, 0:1], NN_G, "bf")
        tf = sb.tile([128, 1], F32, tag="t_tf")
        nc.vector.tensor_tensor(tf[:], sumfg, sfv[:], ALU.mult)
        bfe = sb.tile([128, 1], F32, tag="t_bfe")
        nc.vector.scalar_tensor_tensor(bfe[:], tf[:], -1.0 / NN_G, bnf[:, 1:2],
                                       ALU.mult, ALU.add)

        outsb = sb.tile([128, NN], F32, tag="t_nT")
        half = NN // 2
        nc.scalar.activation(outsb[:, 0:half], h3sb[:, 0:half], ACTF.Identity,
                             bias=bfe[:], scale=sfv[:])
        nc.sync.dma_start(y_d[:, 0:half], outsb[:, 0:half])
        nc.scalar.activation(outsb[:, half:NN], h3sb[:, half:NN], ACTF.Identity,
                             bias=bfe[:], scale=sfv[:])
        nc.sync.dma_start(y_d[:, half:NN], outsb[:, half:NN])

    with tile.TileContext(nc) as tc:
        with (
            tc.tile_pool(name="sb", bufs=1) as sb,
            tc.tile_pool(name="sb2", bufs=2) as sb2,
            tc.tile_pool(name="dram", bufs=1, space="DRAM") as dram,
            tc.tile_pool(name="ps", bufs=2, space="PSUM") as ps,
            tc.tile_pool(name="psone", bufs=1, space="PSUM") as psone,
        ):
            body(tc, sb, sb2, dram, ps, psone)

    nc.compile()
    return nc


def get_nc():
    if "nc" not in _CACHE:
        _CACHE["nc"] = build()
    return _CACHE["nc"]


def make_in_maps(node_attr, edge_attr, W1, Wg, att_src, att_dst,
                 bn0_g, bn0_b, bn1_g, bn1_b, bnf_g, bnf_b):
    node_attr = np.asarray(node_attr, np.float32)
    edge_attr = np.asarray(edge_attr, np.float32)
    nodeT = np.ascontiguousarray(node_attr.T)            # [64, 20000]
    edgeT = np.ascontiguousarray(edge_attr.T)            # [16, 120000]
    W1 = np.ascontiguousarray(np.asarray(W1, np.float32))
    Wg = np.ascontiguousarray(np.asarray(Wg, np.float32))
    va = (Wg @ np.asarray(att_src, np.float32)).astype(np.float32)
    vd = (Wg @ np.asarray(att_dst, np.float32)).astype(np.float32)
    vavd = np.ascontiguousarray(np.stack([va, vd], axis=1))
    bn0p = np.ascontiguousarray(np.stack(
        [np.asarray(bn0_g, np.float32) * EE_G, np.asarray(bn0_b, np.float32)], axis=1))
    bn1p = np.ascontiguousarray(np.stack(
        [np.asarray(bn1_g, np.float32) * EE_G, np.asarray(bn1_b, np.float32)], axis=1))
    bnfp = np.ascontiguousarray(np.stack(
        [np.asarray(bnf_g, np.float32) * NN_G, np.asarray(bnf_b, np.float32)], axis=1))
    in_maps = []
    for c in range(NCORES):
        e0 = c * EE
        ec = edge_attr[e0:e0 + EE]                       # [15000, 16]
        esv = np.ascontiguousarray(
            ec.reshape(8, EE // 8, DE).transpose(0, 2, 1).reshape(128, -1))
        in_maps.append({
            "nT": np.ascontiguousarray(nodeT[:, c * NN:(c + 1) * NN]),
            "eT": np.ascontiguousarray(edgeT[:, e0:e0 + EE]),
            "esv": esv,
            "W1": W1,
            "vavd": vavd,
            "Wg": Wg,
            "bn0": bn0p,
            "bn1": bn1p,
            "bnf": bnfp,
        })
    return in_maps


def _expected_structure(edge_index, index_2step):
    """The deterministic graph from setup_inputs: src = repeat(arange(N), 6),
    line-graph = within-group ordered pairs (no diag) + self loops."""
    src = np.asarray(edge_index)[0]
    if not np.array_equal(src, np.repeat(np.arange(NN_G), DEG)):
        return False
    ii, jj = np.meshgrid(np.arange(DEG), np.arange(DEG), indexing="ij")
    off = ~np.eye(DEG, dtype=bool)
    ii, jj = ii[off], jj[off]
    base = (np.arange(NN_G) * DEG)[:, None]
    s2 = np.concatenate([(base + ii[None, :]).ravel(), np.arange(EE_G)])
    d2 = np.concatenate([(base + jj[None, :]).ravel(), np.arange(EE_G)])
    i2 = np.asarray(index_2step)
    return np.array_equal(i2[0], s2) and np.array_equal(i2[1], d2)


def _numpy_fallback(edge_attr, node_attr, bn0_g, bn0_b, W1, bn1_g, bn1_b,
                    Wg, att_src, att_dst, gat_bias, bnf_g, bnf_b,
                    edge_index, index_2step, num_nodes):
    """Exact host reimplementation of the reference for unexpected graphs."""
    f = np.float32
    ea, na = np.asarray(edge_attr, f), np.asarray(node_attr, f)
    idx = np.asarray(edge_index)
    i2 = np.asarray(index_2step)
    n = int(num_nodes)

    def bn(x, g, b):
        mu = x.mean(0)
        var = x.var(0)
        return (x - mu) / np.sqrt(var + EPS) * np.asarray(g, f) + np.asarray(b, f)

    h0 = np.concatenate([na[idx[0]], ea], 1)
    h1 = np.maximum(bn(bn(h0, bn0_g, bn0_b) @ np.asarray(W1, f), bn1_g, bn1_b), 0)
    x = h1 @ np.asarray(Wg, f)
    a_s = x @ np.asarray(att_src, f)
    a_d = x @ np.asarray(att_dst, f)
    s, d = i2[0], i2[1]
    e = a_s[s] + a_d[d]
    e = np.where(e > 0, e, 0.2 * e)
    m = np.full(x.shape[0], -np.inf, f)
    np.maximum.at(m, d, e)
    ex = np.exp(e - m[d])
    den = np.zeros(x.shape[0], f)
    np.add.at(den, d, ex)
    alpha = ex / (den[d] + 1e-16)
    h2 = np.zeros_like(x)
    np.add.at(h2, d, alpha[:, None] * x[s])
    h2 += np.asarray(gat_bias, f)
    h3 = np.zeros((n, x.shape[1]), f)
    np.add.at(h3, idx[0], h2)
    return bn(h3, bnf_g, bnf_b).astype(np.float32)


def kernel(edge_attr, node_attr, bn0_g, bn0_b, W1, bn1_g, bn1_b,
           Wg, att_src, att_dst, gat_bias, bnf_g, bnf_b,
           edge_index, index_2step, num_nodes):
    """Full inputs in, full [20000, 128] float32 output out."""
    global LAST_RESULTS
    if not _expected_structure(edge_index, index_2step):
        return _numpy_fallback(edge_attr, node_attr, bn0_g, bn0_b, W1, bn1_g,
                               bn1_b, Wg, att_src, att_dst, gat_bias, bnf_g,
                               bnf_b, edge_index, index_2step, num_nodes)
    _install_ntff_hook()
    in_maps = make_in_maps(node_attr, edge_attr, W1, Wg, att_src, att_dst,
                           bn0_g, bn0_b, bn1_g, bn1_b, bnf_g, bnf_b)
    nc = get_nc()
    res = bass_utils.run_bass_kernel_spmd(nc, in_maps, core_ids=list(range(NCORES)))
    LAST_RESULTS = res
    yT = np.concatenate([res.results[c]["y"] for c in range(NCORES)], axis=1)
    return np.ascontiguousarray(yT.T).astype(np.float32)



# revision 39
# speedup vs baseline: 1.2326x; 1.2326x over previous
"""Trainium2 Bass kernel for nn_AttrsEncoderLayers (gnn_message_passing).

Math (from the reference):
  h0 = concat(node_attr[src], edge_attr)        [E, 80]
  h1 = relu(BN1(BN0(h0) @ W1))                  [E, 128]
  x  = h1 @ Wg ; a_src = x@att_src ; a_dst = x@att_dst
  dense 6x6 softmax attention within each node's 6-edge group (incl. self-loop)
  h3[n] = sum_{d in g(n)} sum_s alpha[d,s] x[s]   -> BNf(h3)

Key structure: the first collective on this runtime cannot complete before
~62us (fixed barrier + CC warmup), so ALL global-stat-dependent compute is
deferred behind ONE AllReduce and the dead window is filled with local work:

  - M = sum_e h0 h0^T [80x80] is computed per-core during the window from a
    host-packed tile layout ct=[node|6x16 edges|1] (40 accumulating matmuls,
    no device transposes), exploiting that each node sources exactly 6 edges.
  - ONE AllReduce carries [M | sums]  ->  BN0 scale D = diag stats, and BN1
    scale from the quadratic form var1_j = W1p_j^T (Sg - sm sm^T/E) W1p_j / E.
    BN1 + ReLU then FUSE into the mm1 PSUM eviction (no h1pre pass, no 2nd
    stats collective).
  - rsqrt is computed as exp(-0.5*ln(x)) so the whole kernel uses a single
    activation table (natural_log_exp_and_others) - no table-switch stalls.
  - second (final) AllReduce: BNf stats [128,2].

Per core: 2500 nodes, 15000 edges (shard by source-node blocks).
"""
import sys
import types

for _p in ("/opt/trn_rl_repo", "/root/.axon_site/_ro/trn_rl_repo"):
    if _p not in sys.path:
        sys.path.insert(0, _p)

import numpy as np
import concourse.bass as bass
import concourse.tile as tile
from concourse import bacc, mybir
from concourse import bass_utils

# ---------------------------------------------------------------- constants
NCORES = 8
NN_G, DEG = 20000, 6
EE_G = NN_G * DEG              # 120000
NN = NN_G // NCORES            # 2500 nodes per core
EE = NN * DEG                  # 15000 edges per core
DN, DE, DIN = 64, 16, 80
HID = 128
OUT = 128
EPS = 1e-5
F32 = mybir.dt.float32
F32R = mybir.dt.float32r
ALU = mybir.AluOpType
ACTF = mybir.ActivationFunctionType

NT = 20                        # ct tiles (2500 nodes -> 20 tiles of 128, padded)
CTW = DN + DEG * DE + 2        # 162 cols per ct tile: [node|edges|1|0] (even width)
MM1_CH = 1024                  # mm1 psum chunk (2 bank-aligned matmuls of 512)
AMM_CH = 500                   # a-matmul psum chunk (6 per 3000-edge block)
CMB_CH = 3072                  # combine chunk: 512 groups (last chunk 452)
NPW = 125                      # group-major partitions (20 groups each)
GPP = 20                       # groups per partition in w' layout
RG = [list(range(NCORES))]

_CACHE = {}
LAST_RESULTS = None
import os as _os
KSTAGE = int(_os.environ.get("KSTAGE", "7"))

if not getattr(bass_utils, "_ldwopt_patched", False):
    bass_utils._ldwopt_patched = True
    _orig_walrus_args = bass_utils.get_walrus_args

    def _walrus_args_ldwopt(*a, **k):
        return [x.replace("--enable-ldw-opt=false", "--enable-ldw-opt=true")
                for x in _orig_walrus_args(*a, **k)]

    bass_utils.get_walrus_args = _walrus_args_ldwopt


def _install_ntff_hook():
    """Register the axon NTFF profiling hook under the name bass_utils expects."""
    try:
        import antenv.axon_hooks  # noqa: F401
        return
    except ImportError:
        pass
    try:
        import trn_agent_boot.trn_boot as tb
        hook = tb._ntff_profile_via_ctypes("/opt/axon/libaxon_pjrt.so")
    except Exception:
        hook = None
    mod_antenv = sys.modules.get("antenv") or types.ModuleType("antenv")
    mod_hooks = types.ModuleType("antenv.axon_hooks")
    _reg = {"hook": hook}
    mod_hooks.set_axon_ntff_profile_hook = lambda h: _reg.__setitem__("hook", h)
    mod_hooks.get_axon_ntff_profile_hook = lambda: _reg["hook"]
    mod_antenv.axon_hooks = mod_hooks
    sys.modules.setdefault("antenv", mod_antenv)
    sys.modules["antenv.axon_hooks"] = mod_hooks


def _rsqrt_scale(nc, sb, ssum, ssq, gD, divisor, tag):
    """scale = gD * rsqrt(divisor*ssq - ssum^2 + eps*divisor^2), via exp/ln.

    ssum, ssq, gD: [P,1] tiles (gD = gamma * divisor, host pre-scaled).
    Identity: divisor^2*var = divisor*sumsq - sum^2."""
    P = ssum.shape[0]
    q = sb.tile([P, 1], F32, tag=f"{tag}_q")
    nc.vector.tensor_tensor(q[:], ssum[:], ssum[:], ALU.mult)
    vD2 = sb.tile([P, 1], F32, tag=f"{tag}_v")
    nc.vector.scalar_tensor_tensor(vD2[:], ssq[:], float(divisor), q[:],
                                   ALU.mult, ALU.subtract)
    eps = sb.tile([P, 1], F32, tag=f"{tag}_e")
    nc.vector.memset(eps[:], EPS * divisor * divisor)
    lg = sb.tile([P, 1], F32, tag=f"{tag}_l")
    nc.scalar.activation(lg[:], vD2[:], ACTF.Ln, bias=eps[:])
    rs = sb.tile([P, 1], F32, tag=f"{tag}_r")
    nc.scalar.activation(rs[:], lg[:], ACTF.Exp, scale=-0.5)
    sc = sb.tile([P, 1], F32, tag=f"{tag}_s")
    nc.vector.tensor_tensor(sc[:], gD[:], rs[:], ALU.mult)
    return sc


def build(stage=None):
    if stage is None:
        stage = KSTAGE
    nc = bacc.Bacc("TRN2", target_bir_lowering=False, debug=False,
                   num_devices=NCORES)

    nT_d = nc.dram_tensor("nT", [DN, NN], F32R, kind="ExternalInput").ap()
    eT_d = nc.dram_tensor("eT", [DE, EE], F32R, kind="ExternalInput").ap()
    ct_d = nc.dram_tensor("ct", [128, NT * CTW], F32R, kind="ExternalInput").ap()
    W1_d = nc.dram_tensor("W1", [DIN, HID], F32, kind="ExternalInput").ap()
    vavd_d = nc.dram_tensor("vavd", [HID, 2], F32R, kind="ExternalInput").ap()
    Wg_d = nc.dram_tensor("Wg", [HID, OUT], F32R, kind="ExternalInput").ap()
    id80_d = nc.dram_tensor("id80", [DIN, DIN], F32, kind="ExternalInput").ap()
    bn0_d = nc.dram_tensor("bn0", [DIN, 2], F32, kind="ExternalInput").ap()
    bn1_d = nc.dram_tensor("bn1", [HID, 2], F32, kind="ExternalInput").ap()
    bnf_d = nc.dram_tensor("bnf", [OUT, 2], F32, kind="ExternalInput").ap()
    y_d = nc.dram_tensor("y", [OUT, NN], F32, kind="ExternalOutput").ap()

    def body(tc, sb, sb2, dram, psq, psA, psmn, psp2):
        # h1 allocated first; its pre-mm1-dead space hosts ct and nT views.
        # h0T's post-mm1-dead space hosts a_sb and wline views.
        h1 = sb.tile([128, EE], F32R, tag="t_h1")
        h0T = sb.tile([DIN, EE], F32R, tag="t_h0T")
        NT_OFF = NT * CTW + 80   # 3300: nT view offset inside h1
        # ---------------- loads
        for i in range(4):
            c0 = i * 5 * CTW
            nc.sync.dma_start(h1[:, c0:c0 + 5 * CTW], ct_d[:, c0:c0 + 5 * CTW])
        nc.sync.dma_start(h1[0:DN, NT_OFF:NT_OFF + NN], nT_d)
        nc.sync.dma_start(h0T[DN:DIN, :], eT_d)
        W1 = sb.tile([DIN, HID], F32, tag="t_W1")
        nc.sync.dma_start(W1[:], W1_d)
        vavd = sb.tile([HID, 2], F32R, tag="t_vavd")
        nc.sync.dma_start(vavd[:], vavd_d)
        Wg = sb.tile([HID, OUT], F32R, tag="t_Wg")
        nc.sync.dma_start(Wg[:], Wg_d)
        id80 = sb.tile([DIN, DIN], F32, tag="t_id80")
        nc.sync.dma_start(id80[:], id80_d)
        bn0 = sb.tile([DIN, 2], F32, tag="t_bn0")
        nc.sync.dma_start(bn0[:], bn0_d)
        bn1 = sb.tile([HID, 2], F32, tag="t_bn1")
        nc.sync.dma_start(bn1[:], bn1_d)
        bnf = sb.tile([OUT, 2], F32, tag="t_bnf")
        nc.sync.dma_start(bnf[:], bnf_d)

        ones1 = sb.tile([DIN, 2], F32, tag="t_ones1")
        nc.vector.memset(ones1[:], 1.0)
        ones80 = sb.tile([DIN, 1], F32R, tag="t_ones80")
        nc.vector.tensor_scalar(ones80[:], ones1[:, 0:1], 1.0, None, ALU.mult)
        ones1r = sb.tile([1, 128], F32, tag="t_ones1r")
        nc.vector.memset(ones1r[:], 1.0)
        ones_r = sb.tile([1, 128], F32R, tag="t_onesr")
        nc.vector.tensor_scalar(ones_r[:], ones1r[:], 1.0, None, ALU.mult)

        # ---------------- moment matrices on PE (fills the CC-barrier window)
        # psMn[64,161] = sum_tiles ct[:,0:64]^T @ ct  -> [Mnn/6 | cross(s,f) | nsums]
        # psP2[96,97]  = sum_tiles ct[:,64:160]^T @ ct[:,64:161] -> [P2 | gsums]
        Mn_ps = psmn.tile([128, CTW], F32, tag="psmn")
        for t in range(NT):
            c0 = t * CTW
            nc.tensor.matmul(Mn_ps[0:DN, 0:CTW], h1[:, c0:c0 + DN],
                             h1[:, c0:c0 + CTW], start=(t == 0), stop=(t == NT - 1))
        P2_ps = psp2.tile([128, CTW], F32, tag="psp2")
        for t in range(NT):
            c0 = t * CTW
            nc.tensor.matmul(P2_ps[0:96, 0:98], h1[:, c0 + DN:c0 + 160],
                             h1[:, c0 + DN:c0 + CTW], start=(t == 0), stop=(t == NT - 1))

        mn_sb = sb.tile([DN, CTW], F32, tag="t_mn")
        nc.vector.tensor_copy(mn_sb[:], Mn_ps[0:DN, 0:CTW])
        p2_sb = sb.tile([96, 98], F32, tag="t_p2")
        nc.scalar.copy(p2_sb[:], P2_ps[0:96, 0:98])

        # ---------------- assemble Masm [80, 81] = [M | sums]
        Masm = sb.tile([DIN, 81], F32, tag="t_masm")
        nc.vector.memset(Masm[DN:DIN, :], 0.0)
        # top-left: 6 * sum_n n n^T ; node sums *6 in col 80
        nc.vector.tensor_scalar(Masm[0:DN, 0:DN], mn_sb[:, 0:DN], float(DEG),
                                None, ALU.mult)
        nc.vector.tensor_scalar(Masm[0:DN, 80:81], mn_sb[:, 160:161], float(DEG),
                                None, ALU.mult)
        # top-right: Mne = sum_s cross(s,f)
        nc.vector.tensor_reduce(
            Masm[0:DN, DN:DIN],
            mn_sb[:, DN:160].rearrange("p (s f) -> p f s", s=DEG),
            axis=mybir.AxisListType.X, op=ALU.add)
        # Mee = sum_s diag-block_s(P2)
        mee_st = sb.tile([DE, DEG * DE], F32, tag="t_meest")
        for s in range(DEG):
            nc.sync.dma_start(mee_st[:, s * DE:(s + 1) * DE],
                              p2_sb[s * DE:(s + 1) * DE, s * DE:(s + 1) * DE])
        mee = sb.tile([DE, DE], F32, tag="t_mee")
        nc.vector.tensor_reduce(
            mee[:], mee_st[:].rearrange("p (s f) -> p f s", s=DEG),
            axis=mybir.AxisListType.X, op=ALU.add)
        nc.sync.dma_start(Masm[DN:DIN, DN:DIN], mee[:])
        # edge sums: fold group-sum col of P2 over s
        es6 = sb.tile([DE, DEG], F32, tag="t_es6")
        for s in range(DEG):
            nc.sync.dma_start(es6[:, s:s + 1], p2_sb[s * DE:(s + 1) * DE, 96:97])
        esum = sb.tile([DE, 1], F32, tag="t_esum")
        nc.vector.tensor_reduce(esum[:], es6[:], axis=mybir.AxisListType.X,
                                op=ALU.add)
        nc.sync.dma_start(Masm[DN:DIN, 80:81], esum[:])

        # ---------------- AllReduce #1: [80, 81]
        ar_in = dram.tile([DIN, 81], F32, tag="ar1_in")
        ar_out = dram.tile([DIN, 81], F32, tag="ar1_out")
        nc.sync.dma_start(ar_in[:], Masm[:])
        nc.gpsimd.collective_compute(
            "AllReduce", ALU.add, replica_groups=RG,
            ins=[ar_in[:].opt()], outs=[ar_out[:].opt()])
        Sg = sb.tile([DIN, 81], F32, tag="t_sg")
        nc.sync.dma_start(Sg[:], ar_out[:])

        # node part of h0T: each node column repeated 6x (runs during the window)
        for e0 in range(0, EE, 3000):
            g0, ng = e0 // DEG, 3000 // DEG
            nvrep = h1[0:DN, NT_OFF + g0:NT_OFF + g0 + ng] \
                .unsqueeze(2).broadcast_to([DN, ng, DEG])
            nc.vector.tensor_scalar(
                h0T[0:DN, e0:e0 + 3000].rearrange("p (g s) -> p g s", s=DEG),
                nvrep, 1.0, None, ALU.mult)

        # ---------------- post-AR math: D, W1p, BN1 scale/bias
        sm = Sg[:, 80:81]
        ssq = sb.tile([DIN, 1], F32, tag="t_ssq")
        dscr = sb.tile([DIN, DIN], F32, tag="t_dscr")
        nc.vector.scalar_tensor_tensor(dscr[:], Sg[:, 0:DIN], 1.0,
                                       id80[:], ALU.mult, ALU.mult,
                                       accum_out=ssq[:])
        s0v = _rsqrt_scale(nc, sb, sm, ssq, bn0[:, 0:1], EE_G, "b0")
        W1p = sb.tile([DIN, HID], F32R, tag="t_W1p")
        nc.vector.tensor_scalar(W1p[:], W1[:], s0v[:], None, ALU.mult)

        if stage < 2:
            outsb = sb.tile([128, NN], F32, tag="t_h3")
            nc.vector.memset(outsb[:], 0.0)
            nc.vector.tensor_copy(outsb[0:DIN, 0:1], s0v[:])
            nc.vector.tensor_copy(outsb[0:DIN, 1:2], ssq[:])
            nc.vector.tensor_copy(outsb[0:DIN, 2:81], Sg[:, 2:81])
            nc.sync.dma_start(y_d, outsb[:])
            return

        # Bt = (top-right block of Sg)^T  [16, 64]
        Bt_ps = psp2.tile([128, CTW], F32, tag="psp2")
        nc.tensor.transpose(Bt_ps[0:DE, 0:DN], Sg[0:DN, DN:DIN],
                            id80[0:DN, 0:DN])
        bt_tmp = sb.tile([DE, DN], F32R, tag="t_btt")
        nc.vector.tensor_copy(bt_tmp[:], Bt_ps[0:DE, 0:DN])
        # full symmetric Sg in SBUF: top 64 rows from Sg, bottom-left from Bt
        SgF = sb.tile([DIN, DIN], F32R, tag="t_sgf")
        nc.vector.tensor_scalar(SgF[:], Sg[:, 0:DIN], 1.0, None, ALU.mult)
        nc.sync.dma_start(SgF[DN:DIN, 0:DN], bt_tmp[:])
        # T2 = Sg @ W1p (one matmul; Sg symmetric), V = W1p * T2, q2 = colsum(V)
        SgR = Sg[:].bitcast(F32R)
        T2_ps = psmn.tile([128, CTW], F32, tag="psmn")
        nc.tensor.matmul(T2_ps[0:DIN, 0:HID], SgF[:], W1p[:],
                         start=True, stop=True)
        V = sb.tile([DIN, HID], F32R, tag="t_V")
        nc.vector.tensor_tensor(V[:], W1p[:], T2_ps[0:DIN, 0:HID], ALU.mult)
        q2_ps = psA.tile([2, AMM_CH], F32, tag="psA")
        nc.tensor.matmul(q2_ps[0:1, 0:HID], ones80[:], V[:], start=True, stop=True)
        q2row = sb.tile([1, HID], F32, tag="t_q2r")
        nc.vector.tensor_copy(q2row[:], q2_ps[0:1, 0:HID])
        q2t = sb.tile([HID, 1], F32, tag="t_q2t")
        nc.sync.dma_start(q2t[:], q2row[:])
        r1in = sb.tile([DIN, 2], F32R, tag="t_r1in")
        nc.vector.tensor_copy(r1in[:, 0:1], sm)
        nc.vector.tensor_copy(r1in[:, 1:2], sm)
        r1_ps = psp2.tile([128, CTW], F32, tag="psp2")
        nc.tensor.matmul(r1_ps[0:HID, 158:160], W1p[:], r1in[:],
                         start=True, stop=True)
        r1 = sb.tile([HID, 1], F32, tag="t_r1")
        nc.vector.tensor_copy(r1[:], r1_ps[0:HID, 158:159])
        s1v = _rsqrt_scale(nc, sb, r1, q2t, bn1[:, 0:1], EE_G, "b1")
        t1 = sb.tile([HID, 1], F32, tag="t_t1")
        nc.vector.tensor_tensor(t1[:], r1[:], s1v[:], ALU.mult)
        b1e = sb.tile([HID, 1], F32, tag="t_b1e")
        nc.vector.scalar_tensor_tensor(b1e[:], t1[:], -1.0 / EE_G, bn1[:, 1:2],
                                       ALU.mult, ALU.add)

        # ---------------- mm1 with fused BN1+ReLU eviction
        for e0 in range(0, EE, MM1_CH):
            ch = min(MM1_CH, EE - e0)
            psB = psq.tile([128, MM1_CH], F32, tag="psq")
            for i0 in range(0, ch, 512):
                w_ = min(512, ch - i0)
                nc.tensor.matmul(psB[:, i0:i0 + w_], W1p[:],
                                 h0T[:, e0 + i0:e0 + i0 + w_],
                                 start=True, stop=True)
            nc.scalar.activation(h1[:, e0:e0 + ch], psB[:, 0:ch],
                                 ACTF.Relu, bias=b1e[:], scale=s1v[:])

        if stage < 3:
            outsb = sb.tile([128, NN], F32, tag="t_h3")
            nc.vector.memset(outsb[:], 0.0)
            nc.vector.tensor_reduce(outsb[:, 0:1], h1[:, 0:3000].bitcast(F32),
                                    axis=mybir.AxisListType.X, op=ALU.add)
            nc.sync.dma_start(y_d, outsb[:])
            return

        # ---------------- a-matmul; copies split vector/gpsimd; DMA-fold
        # a_sb lives in dead h0T rows 0:2
        for k, a0 in enumerate(range(0, EE, AMM_CH)):
            psa = psA.tile([2, AMM_CH], F32, tag="psA")
            nc.tensor.matmul(psa[:], vavd[:], h1[:, a0:a0 + AMM_CH],
                             start=True, stop=True)
            if k % 2 == 0:
                nc.vector.tensor_copy(h0T[0:2, a0:a0 + AMM_CH], psa[:])
            else:
                nc.scalar.copy(h0T[0:2, a0:a0 + AMM_CH], psa[:])
        asrc = sb.tile([NPW, GPP * DEG], F32, tag="t_asrc")
        adst = sb.tile([NPW, GPP * DEG], F32, tag="t_adst")
        NPB = 3000 // (GPP * DEG)   # 25 partitions per 3000-edge block
        for bi, e0 in enumerate(range(0, EE, 3000)):
            p0 = bi * NPB
            nc.sync.dma_start(asrc[p0:p0 + NPB, :],
                              h0T[0:1, e0:e0 + 3000].bitcast(F32))
            nc.sync.dma_start(adst[p0:p0 + NPB, :],
                              h0T[1:2, e0:e0 + 3000].bitcast(F32))

        if stage < 4:
            outsb = sb.tile([128, NN], F32, tag="t_h3")
            nc.vector.memset(outsb[:], 0.0)
            nc.vector.tensor_reduce(outsb[0:NPW, 0:1], asrc[:],
                                    axis=mybir.AxisListType.X, op=ALU.add)
            nc.vector.tensor_reduce(outsb[0:NPW, 1:2], adst[:],
                                    axis=mybir.AxisListType.X, op=ALU.add)
            nc.sync.dma_start(y_d, outsb[:])
            return

        # ---------------- dense 6x6 group attention -> per-edge weight w
        L = sb.tile([NPW, GPP * 36], F32, tag="t_L")
        asrc_v = asrc[:].rearrange("p (t s) -> p t s", s=DEG).unsqueeze(2) \
            .broadcast_to([NPW, GPP, DEG, DEG])
        adst_v = adst[:].rearrange("p (t d) -> p t d", d=DEG).unsqueeze(3) \
            .broadcast_to([NPW, GPP, DEG, DEG])
        nc.vector.tensor_tensor(
            L[:].rearrange("p (t d s) -> p t d s", d=DEG, s=DEG),
            asrc_v, adst_v, ALU.add)
        nc.vector.scalar_tensor_tensor(L[:], L[:], 0.2, L[:], ALU.mult, ALU.max)
        nc.scalar.activation(L[:], L[:], ACTF.Exp)
        R = sb.tile([NPW, GPP * DEG], F32, tag="t_R")
        nc.vector.tensor_reduce(
            R[:], L[:].rearrange("p (t d s) -> p t d s", d=DEG, s=DEG),
            axis=mybir.AxisListType.X, op=ALU.add)
        Rinv = sb.tile([NPW, GPP * DEG], F32, tag="t_Rinv")
        nc.vector.reciprocal(Rinv[:], R[:])
        Q = sb.tile([NPW, GPP * 36], F32, tag="t_Q")
        rinv_v = Rinv[:].rearrange("p (t d) -> p t d", d=DEG).unsqueeze(2) \
            .broadcast_to([NPW, GPP, DEG, DEG])
        nc.vector.tensor_tensor(
            Q[:].rearrange("p (t s d) -> p t s d", s=DEG, d=DEG),
            L[:].rearrange("p (t d s) -> p t s d", d=DEG, s=DEG),
            rinv_v, ALU.mult)
        wp = sb.tile([NPW, GPP * DEG], F32, tag="t_wp")
        nc.vector.tensor_reduce(
            wp[:], Q[:].rearrange("p (t s d) -> p t s d", s=DEG, d=DEG),
            axis=mybir.AxisListType.X, op=ALU.add)

        if stage < 5:
            outsb = sb.tile([128, NN], F32, tag="t_h3")
            nc.vector.memset(outsb[:], 0.0)
            nc.vector.tensor_reduce(outsb[0:NPW, 0:1], wp[:],
                                    axis=mybir.AxisListType.X, op=ALU.add)
            nc.sync.dma_start(y_d, outsb[:])
            return

        # ---------------- combine: h3 = Wg^T sum_s (w * h1) per group
        n_cmb = (EE + CMB_CH - 1) // CMB_CH
        accF = sb.tile([128, n_cmb], F32, tag="t_accF")
        accFq = sb.tile([128, n_cmb], F32, tag="t_accFq")
        h3sb = sb.tile([128, NN], F32, tag="t_h3")
        # wline lives in h0T row 0 (a_sb is dead once attention is done)
        for bi in range(5):
            p0 = bi * NPB
            nc.sync.dma_start(h0T[0:1, bi * 3000:(bi + 1) * 3000].bitcast(F32),
                              wp[p0:p0 + NPB, :])
        for c in range(n_cmb):
            e0 = c * CMB_CH
            ch = min(CMB_CH, EE - e0)
            wh1 = sb2.tile([128, CMB_CH], F32R, tag="t_wh1")
            # broadcast w to 128 partitions via ones-matmul (PE), multiply
            for j, m0 in enumerate(range(e0, e0 + ch, MM1_CH)):
                mw = min(MM1_CH, e0 + ch - m0)
                psW = psq.tile([128, MM1_CH], F32, tag="psq")
                for i0 in range(0, mw, 512):
                    w_ = min(512, mw - i0)
                    nc.tensor.matmul(psW[:, i0:i0 + w_], ones_r[:],
                                     h0T[0:1, m0 + i0:m0 + i0 + w_],
                                     start=True, stop=True)
                nc.vector.tensor_tensor(wh1[:, m0 - e0:m0 - e0 + mw],
                                        h1[:, m0:m0 + mw],
                                        psW[:, 0:mw].bitcast(F32R), ALU.mult)
            ngr = ch // DEG
            psH = psq.tile([128, MM1_CH], F32, tag="psq")
            wv = wh1[:].rearrange("p (g s) -> p s g", s=DEG)
            for s in range(DEG):
                nc.tensor.matmul(psH[:, 0:ngr], Wg[:], wv[:, s, 0:ngr],
                                 start=(s == 0), stop=(s == DEG - 1))
            g0 = e0 // DEG
            nc.scalar.activation(h3sb[:, g0:g0 + ngr], psH[:, 0:ngr],
                                 ACTF.Copy, accum_out=accF[:, c:c + 1])
            scr2 = sb2.tile([128, CMB_CH // DEG], F32, tag="t_scr2")
            nc.vector.scalar_tensor_tensor(
                scr2[:, 0:ngr], h3sb[:, g0:g0 + ngr], 1.0, h3sb[:, g0:g0 + ngr],
                ALU.mult, ALU.mult, accum_out=accFq[:, c:c + 1])

        if stage < 6:
            outsb = sb.tile([128, NN], F32, tag="t_h3x")
            nc.vector.memset(outsb[:], 0.0)
            nc.vector.tensor_copy(outsb[:, 0:n_cmb], accF[:])
            nc.sync.dma_start(y_d, outsb[:])
            return

        # ---------------- BNf: AllReduce [128,2], scale/bias, apply (vector)
        sumf = sb.tile([128, 2], F32, tag="t_sumf")
        nc.vector.tensor_reduce(sumf[:, 0:1], accF[:], axis=mybir.AxisListType.X,
                                op=ALU.add)
        nc.vector.tensor_reduce(sumf[:, 1:2], accFq[:], axis=mybir.AxisListType.X,
                                op=ALU.add)
        arf_in = dram.tile([OUT, 2], F32, tag="arf_in")
        arf_out = dram.tile([OUT, 2], F32, tag="arf_out")
        nc.sync.dma_start(arf_in[:], sumf[:])
        nc.gpsimd.collective_compute(
            "AllReduce", ALU.add, replica_groups=RG,
            ins=[arf_in[:].opt()], outs=[arf_out[:].opt()])
        Sf = sb.tile([OUT, 2], F32, tag="t_Sf")
        nc.sync.dma_start(Sf[:], arf_out[:])
        sfv = _rsqrt_scale(nc, sb, Sf[:, 0:1], Sf[:, 1:2], bnf[:, 0:1], NN_G, "bf")
        tf = sb.tile([OUT, 1], F32, tag="t_tf")
        nc.vector.tensor_tensor(tf[:], Sf[:, 0:1], sfv[:], ALU.mult)
        bfe = sb.tile([OUT, 1], F32, tag="t_bfe")
        nc.vector.scalar_tensor_tensor(bfe[:], tf[:], -1.0 / NN_G, bnf[:, 1:2],
                                       ALU.mult, ALU.add)
        half = NN // 2
        nc.vector.tensor_scalar(h3sb[:, 0:half], h3sb[:, 0:half], sfv[:],
                                bfe[:], ALU.mult, ALU.add)
        nc.sync.dma_start(y_d[:, 0:half], h3sb[:, 0:half])
        nc.vector.tensor_scalar(h3sb[:, half:NN], h3sb[:, half:NN], sfv[:],
                                bfe[:], ALU.mult, ALU.add)
        nc.sync.dma_start(y_d[:, half:NN], h3sb[:, half:NN])

    with tile.TileContext(nc) as tc:
        with (
            tc.tile_pool(name="sb", bufs=1) as sb,
            tc.tile_pool(name="sb2", bufs=2) as sb2,
            tc.tile_pool(name="dram", bufs=1, space="DRAM") as dram,
            tc.tile_pool(name="psq", bufs=2, space="PSUM") as psq,
            tc.tile_pool(name="psA", bufs=2, space="PSUM") as psA,
            tc.tile_pool(name="psmn", bufs=1, space="PSUM") as psmn,
            tc.tile_pool(name="psp2", bufs=1, space="PSUM") as psp2,
        ):
            body(tc, sb, sb2, dram, psq, psA, psmn, psp2)

    nc.compile()
    return nc


def get_nc():
    if "nc" not in _CACHE:
        _CACHE["nc"] = build()
    return _CACHE["nc"]


def make_in_maps(node_attr, edge_attr, W1, Wg, att_src, att_dst,
                 bn0_g, bn0_b, bn1_g, bn1_b, bnf_g, bnf_b):
    node_attr = np.asarray(node_attr, np.float32)
    edge_attr = np.asarray(edge_attr, np.float32)
    nodeT = np.ascontiguousarray(node_attr.T)            # [64, 20000]
    edgeT = np.ascontiguousarray(edge_attr.T)            # [16, 120000]
    W1 = np.ascontiguousarray(np.asarray(W1, np.float32))
    Wg = np.ascontiguousarray(np.asarray(Wg, np.float32))
    va = (Wg @ np.asarray(att_src, np.float32)).astype(np.float32)
    vd = (Wg @ np.asarray(att_dst, np.float32)).astype(np.float32)
    vavd = np.ascontiguousarray(np.stack([va, vd], axis=1))
    id80 = np.eye(DIN, dtype=np.float32)
    bn0p = np.ascontiguousarray(np.stack(
        [np.asarray(bn0_g, np.float32) * EE_G, np.asarray(bn0_b, np.float32)], axis=1))
    bn1p = np.ascontiguousarray(np.stack(
        [np.asarray(bn1_g, np.float32) * EE_G, np.asarray(bn1_b, np.float32)], axis=1))
    bnfp = np.ascontiguousarray(np.stack(
        [np.asarray(bnf_g, np.float32) * NN_G, np.asarray(bnf_b, np.float32)], axis=1))
    in_maps = []
    for c in range(NCORES):
        n0, e0 = c * NN, c * EE
        nodes = node_attr[n0:n0 + NN]                    # [2500, 64]
        eg = edge_attr[e0:e0 + EE].reshape(NN, DEG * DE)  # [2500, 96]
        ones = np.ones((NN, 1), np.float32)
        zer = np.zeros((NN, 1), np.float32)
        ct = np.concatenate([nodes, eg, ones, zer], axis=1)  # [2500, 162]
        ct = np.concatenate(
            [ct, np.zeros((NT * 128 - NN, CTW), np.float32)], axis=0)
        ct = np.ascontiguousarray(
            ct.reshape(NT, 128, CTW).transpose(1, 0, 2).reshape(128, NT * CTW))
        in_maps.append({
            "nT": np.ascontiguousarray(nodeT[:, n0:n0 + NN]),
            "eT": np.ascontiguousarray(edgeT[:, e0:e0 + EE]),
            "ct": ct,
            "W1": W1,
            "vavd": vavd,
            "Wg": Wg,
            "id80": id80,
            "bn0": bn0p,
            "bn1": bn1p,
            "bnf": bnfp,
        })
    return in_maps


def _expected_structure(edge_index, index_2step):
    """The deterministic graph from setup_inputs: src = repeat(arange(N), 6),
    line-graph = within-group ordered pairs (no diag) + self loops."""
    src = np.asarray(edge_index)[0]
    if not np.array_equal(src, np.repeat(np.arange(NN_G), DEG)):
        return False
    ii, jj = np.meshgrid(np.arange(DEG), np.arange(DEG), indexing="ij")
    off = ~np.eye(DEG, dtype=bool)
    ii, jj = ii[off], jj[off]
    base = (np.arange(NN_G) * DEG)[:, None]
    s2 = np.concatenate([(base + ii[None, :]).ravel(), np.arange(EE_G)])
    d2 = np.concatenate([(base + jj[None, :]).ravel(), np.arange(EE_G)])
    i2 = np.asarray(index_2step)
    return np.array_equal(i2[0], s2) and np.array_equal(i2[1], d2)


def _numpy_fallback(edge_attr, node_attr, bn0_g, bn0_b, W1, bn1_g, bn1_b,
                    Wg, att_src, att_dst, gat_bias, bnf_g, bnf_b,
                    edge_index, index_2step, num_nodes):
    """Exact host reimplementation of the reference for unexpected graphs."""
    f = np.float32
    ea, na = np.asarray(edge_attr, f), np.asarray(node_attr, f)
    idx = np.asarray(edge_index)
    i2 = np.asarray(index_2step)
    n = int(num_nodes)

    def bn(x, g, b):
        mu = x.mean(0)
        var = x.var(0)
        return (x - mu) / np.sqrt(var + EPS) * np.asarray(g, f) + np.asarray(b, f)

    h0 = np.concatenate([na[idx[0]], ea], 1)
    h1 = np.maximum(bn(bn(h0, bn0_g, bn0_b) @ np.asarray(W1, f), bn1_g, bn1_b), 0)
    x = h1 @ np.asarray(Wg, f)
    a_s = x @ np.asarray(att_src, f)
    a_d = x @ np.asarray(att_dst, f)
    s, d = i2[0], i2[1]
    e = a_s[s] + a_d[d]
    e = np.where(e > 0, e, 0.2 * e)
    m = np.full(x.shape[0], -np.inf, f)
    np.maximum.at(m, d, e)
    ex = np.exp(e - m[d])
    den = np.zeros(x.shape[0], f)
    np.add.at(den, d, ex)
    alpha = ex / (den[d] + 1e-16)
    h2 = np.zeros_like(x)
    np.add.at(h2, d, alpha[:, None] * x[s])
    h2 += np.asarray(gat_bias, f)
    h3 = np.zeros((n, x.shape[1]), f)
    np.add.at(h3, idx[0], h2)
    return bn(h3, bnf_g, bnf_b).astype(np.float32)


def kernel(edge_attr, node_attr, bn0_g, bn0_b, W1, bn1_g, bn1_b,
           Wg, att_src, att_dst, gat_bias, bnf_g, bnf_b,
           edge_index, index_2step, num_nodes):
    """Full inputs in, full [20000, 128] float32 output out."""
    global LAST_RESULTS
    if not _expected_structure(edge_index, index_2step):
        return _numpy_fallback(edge_attr, node_attr, bn0_g, bn0_b, W1, bn1_g,
                               bn1_b, Wg, att_src, att_dst, gat_bias, bnf_g,
                               bnf_b, edge_index, index_2step, num_nodes)
    _install_ntff_hook()
    in_maps = make_in_maps(node_attr, edge_attr, W1, Wg, att_src, att_dst,
                           bn0_g, bn0_b, bn1_g, bn1_b, bnf_g, bnf_b)
    nc = get_nc()
    res = bass_utils.run_bass_kernel_spmd(nc, in_maps, core_ids=list(range(NCORES)))
    LAST_RESULTS = res
    yT = np.concatenate([res.results[c]["y"] for c in range(NCORES)], axis=1)
    return np.ascontiguousarray(yT.T).astype(np.float32)
